# revision 39
# baseline (speedup 1.0000x reference)
"""Trainium2 Bass kernel: 16-head MHA (B=2, S=2048, E=1024) on 8 NeuronCores.

Sharding: core c = (batch b = c // 4, head-group g = c % 4); each core runs
4 heads of one batch (data parallel on B x tensor parallel on heads).  The
output projection is row-sharded: each core produces a partial [S, E] bf16
output; the host sums the 4 head-group partials per batch (f32) and adds bo.

Device pipeline per core (all matmul operands bf16, fp32 PSUM accumulation).
The PE instruction stream is one continuous sequence ordered to never starve
(matmuls back-to-back keep the PE at its top p-state):

  k-proj -> q-proj -> scores(chunk0, both pairs)      [pre-issued: fills the
                                                       xv DMA-arrival gap]
  -> v[n-tiles 0:4] -> attnV(chunk0) -> v[4:8]
  -> attn(c1,p0) -> v[8:12] -> outproj(c0) -> attn(c1,p1) -> v[12:16]
  -> attn(c2,p0) -> outproj(c1) -> attn(c2,p1)
  -> attn(c3,p0) -> outproj(c2) -> attn(c3,p1) -> outproj(c3)

All input DMAs issue on the sync queue in priority order (wk/xk per e-tile
interleaved, then wqv, xq, xv) so each projection's operands stream in just
ahead of the PE.  v is produced directly in [n, dv] layout (stationary x
tile, moving wv) -- no PE transposes.  The softmax denominator rides as a
ones-column in v_aug; normalization bounces the denominator row through DRAM
(reshape to [128, 8] for a wide reciprocal, broadcast back with a 0-stride
partition AP), batched over both head-halves per group.
"""

import numpy as np
import ml_dtypes

B, S, E = 2, 2048, 1024
H, DK = 16, 64
NCORES = 8
G = 4                 # head-groups (tensor parallel degree)
NH = H // G           # heads per core = 4
DKH = NH * DK         # 256 head dims per core
P = 128
MC = 512              # m-chunk (psum bank width in f32)
NMC = S // MC         # 4 m-chunks
NT = S // P           # 16 n-tiles (and m-tiles)
ET = E // P           # 8 e-tiles
PAIRS = NH // 2       # 2 head pairs per core
BF16 = ml_dtypes.bfloat16
SCALE = float(1.0 / np.sqrt(np.float32(DK)))


def _build_program(chunk_ntiles, causal, bias_qk, bias_v):
    """Build the (SPMD, shared across all 8 cores) Bass program.

    chunk_ntiles[c] = number of 128-wide n-tiles to process for m-chunk c.
    causal: apply diagonal-tile masking (memset + tri multiply).
    """
    from contextlib import ExitStack

    import concourse.bass as bass
    import concourse.tile as tile
    from concourse import bacc, mybir
    from concourse.masks import make_identity

    f32 = mybir.dt.float32
    bf16 = mybir.dt.bfloat16
    Exp = mybir.ActivationFunctionType.Exp

    nc = bacc.Bacc(
        "TRN2",
        target_bir_lowering=False,
        debug=False,
        enable_asserts=False,
        num_devices=NCORES,
    )

    # ---- DRAM I/O ----
    xqT = nc.dram_tensor("xqT", [E, S], bf16, kind="ExternalInput").ap()
    xkT = nc.dram_tensor("xkT", [E, S], bf16, kind="ExternalInput").ap()
    xvT = nc.dram_tensor("xvT", [E, S], bf16, kind="ExternalInput").ap()
    wkT = nc.dram_tensor("wkT", [E, DKH], bf16, kind="ExternalInput").ap()
    wqT = nc.dram_tensor("wqT", [E, DKH], bf16, kind="ExternalInput").ap()
    wvT = nc.dram_tensor("wvT", [E, DKH], bf16, kind="ExternalInput").ap()
    woT = nc.dram_tensor("woT", [DKH, E], bf16, kind="ExternalInput").ap()
    dmask = nc.dram_tensor("dmask", [P, P], bf16, kind="ExternalInput").ap()
    if bias_qk:
        bqd = nc.dram_tensor("bq", [DKH, 1], f32, kind="ExternalInput").ap()
        bkd = nc.dram_tensor("bk", [DKH, 1], f32, kind="ExternalInput").ap()
    if bias_v:
        bvd = nc.dram_tensor("bv", [1, DKH], f32, kind="ExternalInput").ap()
    out = nc.dram_tensor("out", [S, E], bf16, kind="ExternalOutput").ap()

    with tile.TileContext(nc) as tc, ExitStack() as ctx:
        const = ctx.enter_context(tc.tile_pool(name="const", bufs=1))
        xpool = ctx.enter_context(tc.tile_pool(name="xpool", bufs=1))
        wpool = ctx.enter_context(tc.tile_pool(name="wpool", bufs=1))
        qkpool = ctx.enter_context(tc.tile_pool(name="qkpool", bufs=1))
        vpool = ctx.enter_context(tc.tile_pool(name="vpool", bufs=1))
        prpool = ctx.enter_context(tc.tile_pool(name="prpool", bufs=18))
        rcpool = ctx.enter_context(tc.tile_pool(name="rcpool", bufs=4))
        bcpool = ctx.enter_context(tc.tile_pool(name="bcpool", bufs=2))
        otpool = ctx.enter_context(tc.tile_pool(name="otpool", bufs=1))
        ostpool = ctx.enter_context(tc.tile_pool(name="ostpool", bufs=3))

        # ---- all input DMAs on the sync queue, in priority order ----
        # (one issuing queue => transfers start in issue order, so each
        # projection's operands arrive just ahead of the PE's need: the DMA
        # engines drain the queue FIFO at full HBM bandwidth.)
        wk_sb = wpool.tile([P, ET, DKH], bf16, tag="wk")
        wq_sb = wpool.tile([P, ET, DKH], bf16, tag="wq")
        wv_sb = wpool.tile([P, ET, DKH], bf16, tag="wv")
        wo_sb = wpool.tile([P, PAIRS, E], bf16, tag="wo")
        xk_sb = xpool.tile([P, ET, S], bf16, tag="xk")
        xq_sb = xpool.tile([P, ET, S], bf16, tag="xq")
        xv_sb = xpool.tile([P, ET, S], bf16, tag="xv")
        xkr = xkT.rearrange("(t p) s -> p t s", p=P)
        xqr = xqT.rearrange("(t p) s -> p t s", p=P)
        xvr = xvT.rearrange("(t p) s -> p t s", p=P)
        wkr = wkT.rearrange("(t p) d -> p t d", p=P)
        wqr = wqT.rearrange("(t p) d -> p t d", p=P)
        wvr = wvT.rearrange("(t p) d -> p t d", p=P)
        for i in range(ET):
            nc.sync.dma_start(out=wk_sb[:, i, :], in_=wkr[:, i, :])
            if i == 0:
                # split so the first projection matmul's [0:512] region
                # dependency fires after a quarter of the bytes
                nc.sync.dma_start(out=xk_sb[:, 0, 0:MC], in_=xkr[:, 0, 0:MC])
                nc.sync.dma_start(out=xk_sb[:, 0, MC:S], in_=xkr[:, 0, MC:S])
            else:
                nc.sync.dma_start(out=xk_sb[:, i, :], in_=xkr[:, i, :])
        for i in range(ET):
            nc.sync.dma_start(out=wq_sb[:, i, :], in_=wqr[:, i, :])
            if i == ET - 1:
                # the last e-tile gates the whole q projection: split it so
                # the first m-chunk's accumulation can finish sooner
                nc.sync.dma_start(out=xq_sb[:, i, 0:MC], in_=xqr[:, i, 0:MC])
                nc.sync.dma_start(out=xq_sb[:, i, MC:S], in_=xqr[:, i, MC:S])
            else:
                nc.sync.dma_start(out=xq_sb[:, i, :], in_=xqr[:, i, :])
        nc.sync.dma_start(out=wv_sb, in_=wvr)
        for i in range(ET):
            nc.sync.dma_start(out=xv_sb[:, i, :], in_=xvr[:, i, :])
        for p in range(PAIRS):
            nc.sync.dma_start(out=wo_sb[:, p, :], in_=woT[P * p : P * (p + 1), :])
        dmask_sb = const.tile([P, P], bf16, tag="dmask")
        nc.sync.dma_start(out=dmask_sb, in_=dmask)
        ident_f32 = const.tile([P, P], f32, tag="ident")
        make_identity(nc, ident_f32)
        if bias_qk:
            bq_sb = const.tile([P, PAIRS], f32, tag="bq")
            nc.sync.dma_start(out=bq_sb, in_=bqd.rearrange("(t p) o -> p (t o)", p=P))
            bk_sb = const.tile([P, PAIRS], f32, tag="bk")
            nc.sync.dma_start(out=bk_sb, in_=bkd.rearrange("(t p) o -> p (t o)", p=P))
        if bias_v:
            bv_sb = const.tile([P, 2 * DKH], f32, tag="bv")
            nc.sync.dma_start(
                out=bv_sb,
                in_=bass.AP(
                    tensor=bvd.tensor,
                    offset=bvd.offset,
                    ap=[[0, P], [1, DKH]],
                ),
            )

        # persistent activation tiles
        qT_sb = [qkpool.tile([P, S], bf16, tag=f"qT{p}", name=f"qT_sb{p}") for p in range(PAIRS)]
        kT_sb = [qkpool.tile([P, S], bf16, tag=f"kT{p}", name=f"kT_sb{p}") for p in range(PAIRS)]
        vaug_sb = [vpool.tile([P, NH, DK + 1], bf16, tag=f"va{j}", name=f"vaug_sb{j}") for j in range(NT)]
        oT_sb = [otpool.tile([P, S], bf16, tag=f"oT{p}", name=f"oT_sb{p}") for p in range(PAIRS)]

        # PE p-state warm-up: the tensor engine's clock ramps only while
        # continuously busy, so burn the DMA-arrival wait on dummy
        # transposes to hit full clock by the first real matmul
        with tc.tile_pool(name="warm_ps", bufs=1, space="PSUM") as wps:
            wt = wps.tile([P, P], f32, tag="warm", name="warm_t")
            for _ in range(33):
                nc.tensor.transpose(wt, ident_f32, ident_f32)

        # ---- stage 1: q/k projections (weight-tile stationary) ----
        # both pairs interleaved per e-tile (8 psum banks) so a single
        # e-tile's DMA arrival unblocks 8 back-to-back matmuls
        with tc.tile_pool(name="pj_ps", bufs=8, space="PSUM") as pjps:
            for dst, w_sb, x_sb, bias in (
                (kT_sb, wk_sb, xk_sb, bk_sb if bias_qk else None),
                (qT_sb, wq_sb, xq_sb, bq_sb if bias_qk else None),
            ):
                ps = [
                    [pjps.tile([P, MC], f32, tag="qk", name="ps_qk") for _ in range(NMC)]
                    for _ in range(PAIRS)
                ]
                for i in range(ET):
                    for p in range(PAIRS):
                        for c in range(NMC):
                            nc.tensor.matmul(
                                ps[p][c],
                                w_sb[:, i, P * p : P * (p + 1)],
                                x_sb[:, i, MC * c : MC * (c + 1)],
                                start=(i == 0),
                                stop=(i == ET - 1),
                            )
                # evict on DVE (idle in this phase) so the ACT engine can
                # start the pre-issued score exps as soon as they appear
                for p in range(PAIRS):
                    for c in range(NMC):
                        dslice = dst[p][:, MC * c : MC * (c + 1)]
                        if bias is not None:
                            nc.vector.tensor_scalar_add(dslice, ps[p][c], bias[:, p : p + 1])
                        else:
                            nc.vector.tensor_copy(dslice, ps[p][c])

        # ---- stages 2+3 pools: 4 (scores) + 2 (o_aug) + 2 (v-proj/outproj)
        # psum banks = 8 total.  v-projection n-tile blocks, attention groups
        # and per-chunk output projections are interleaved in ONE PE stream.
        with (
            tc.tile_pool(name="sc_ps", bufs=2, space="PSUM") as scps,
            tc.tile_pool(name="oa_ps", bufs=2, space="PSUM") as oaps,
            tc.tile_pool(name="mp_ps", bufs=2, space="PSUM") as mpps,
            tc.tile_pool(name="rc_dram", bufs=4, space="DRAM") as rcdram,
        ):

            def emit_vproj(j0, j1):
                # v directly in [n, dv] layout: stationary x-tile [e, n],
                # moving wv [e, dv] accumulated over e-tiles.
                for j in range(j0, j1):
                    ps = mpps.tile([P, MC], f32, tag="mp", name="ps_v")
                    for i in range(ET):
                        nc.tensor.matmul(
                            ps[:, 0:DKH],
                            xv_sb[:, i, P * j : P * (j + 1)],
                            wv_sb[:, i, :],
                            start=(i == 0),
                            stop=(i == ET - 1),
                        )
                    ps3 = ps[:, 0:DKH].rearrange("n (h d) -> n h d", h=NH)
                    if bias_v:
                        bv3 = bv_sb.rearrange("n (h d) -> n h d", h=NH)
                        nc.vector.tensor_add(vaug_sb[j][:, :, 0:DK], ps3, bv3)
                    else:
                        nc.vector.tensor_copy(vaug_sb[j][:, :, 0:DK], ps3)
                    nc.vector.memset(vaug_sb[j][:, :, DK : DK + 1], 1.0)

            class Group:
                """Attention for one (m-chunk c, head-pair p)."""

                def __init__(self, c, p):
                    self.c, self.p = c, p
                    self.J = chunk_ntiles[c]
                    self.probs = [None] * self.J
                    self.oaug = None

                def emit_scores(self, j):
                    c, p = self.c, self.p
                    off = P * (j - 4 * c) if (causal and j >= 4 * c) else 0
                    sc = scps.tile([P, 2 * MC], f32, tag="sc", name="sc_ps_t")
                    for h01 in range(2):
                        nc.tensor.matmul(
                            sc[:, MC * h01 + off : MC * (h01 + 1)],
                            kT_sb[p][64 * h01 : 64 * (h01 + 1), P * j : P * (j + 1)],
                            qT_sb[p][64 * h01 : 64 * (h01 + 1), MC * c + off : MC * (c + 1)],
                            start=True,
                            stop=True,
                        )
                    probs = prpool.tile([P, 2 * MC], bf16, tag="probs", name="probs_t")
                    sc3 = sc.rearrange("p (u m) -> p u m", u=2)
                    pr3 = probs.rearrange("p (u m) -> p u m", u=2)
                    nc.scalar.activation(
                        pr3[:, :, off:MC], sc3[:, :, off:MC], Exp, bias=0.0, scale=SCALE
                    )
                    if causal and j >= 4 * c:
                        for h01 in range(2):
                            base = MC * h01 + off
                            nc.gpsimd.tensor_mul(
                                probs[:, base : base + P],
                                probs[:, base : base + P],
                                dmask_sb,
                            )
                    self.probs[j] = (probs, off)

                def emit_attnv(self, j):
                    if self.oaug is None:
                        self.oaug = [
                            oaps.tile([P, MC], f32, tag="oaug", name=f"oaug{h01}")
                            for h01 in range(2)
                        ]
                    probs, off = self.probs[j]
                    for h01 in range(2):
                        h = 2 * self.p + h01
                        nc.tensor.matmul(
                            self.oaug[h01][0 : DK + 1, off:MC],
                            vaug_sb[j][:, h, :],
                            probs[:, MC * h01 + off : MC * (h01 + 1)],
                            start=(j == 0),
                            stop=(j == self.J - 1),
                        )

                def emit_norm(self):
                    # evict o_aug to SBUF (frees the PSUM slot), then divide
                    # rows 0:64 by row 64 (the ridden-along softmax denom).
                    # The [1, 2*MC] denom row would be a single-lane DVE op;
                    # bounce through DRAM: reshape to [128, 8] for a wide
                    # reciprocal, then re-read with a 0-stride partition AP
                    # as a [64, 2*MC] broadcast.  Both head-halves batched.
                    c, p = self.c, self.p
                    osb = []
                    for h01 in range(2):
                        o = rcpool.tile([DK + 1, MC], f32, tag="osb", name="osb_t")
                        nc.vector.tensor_copy(o, self.oaug[h01][0 : DK + 1, :])
                        osb.append(o)
                    den_d = rcdram.tile([2, MC], f32, tag="den_d", name="den_d_t")
                    for h01 in range(2):
                        nc.sync.dma_start(
                            out=den_d[h01 : h01 + 1, :], in_=osb[h01][DK : DK + 1, :]
                        )
                    den_q = rcpool.tile([P, 2 * MC // P], f32, tag="den_q", name="den_q_t")
                    nc.sync.dma_start(
                        out=den_q,
                        in_=bass.AP(
                            tensor=den_d.tensor,
                            offset=den_d.offset,
                            ap=[[2 * MC // P, P], [1, 2 * MC // P]],
                        ),
                    )
                    rcq = rcpool.tile([P, 2 * MC // P], f32, tag="rcq", name="rcq_t")
                    nc.vector.reciprocal(rcq, den_q)
                    rcd = rcdram.tile([1, 2 * MC], f32, tag="rcd", name="rcd_t")
                    nc.sync.dma_start(
                        out=bass.AP(
                            tensor=rcd.tensor,
                            offset=rcd.offset,
                            ap=[[2 * MC // P, P], [1, 2 * MC // P]],
                        ),
                        in_=rcq,
                    )
                    bc = bcpool.tile([64, 2 * MC], f32, tag="bc", name="bc_t")
                    nc.sync.dma_start(
                        out=bc,
                        in_=bass.AP(
                            tensor=rcd.tensor,
                            offset=rcd.offset,
                            ap=[[0, 64]] + [list(a) for a in rcd.ap[1:]],
                        ),
                    )
                    for h01 in range(2):
                        nc.vector.tensor_mul(
                            oT_sb[p][64 * h01 : 64 * (h01 + 1), MC * c : MC * (c + 1)],
                            osb[h01][0:DK, :],
                            bc[:, MC * h01 : MC * (h01 + 1)],
                        )

                def emit_norm_transpose(self):
                    # all-on-chip normalization (no DMA hops): PE-transpose
                    # each [65, 128] block of o_aug so the denominator becomes
                    # a [128, 1] column, wide-reciprocal it, per-partition
                    # multiply, transpose back.  Used for the final group,
                    # whose normalization latency is tail-exposed; the final
                    # chunk's outproj t-tiles slot between the forward and
                    # back transposes as PE filler for the DVE latency.
                    c, p = self.c, self.p
                    osb = []
                    for h01 in range(2):
                        o = rcpool.tile([DK + 1, MC], f32, tag="osb", name="osb_t")
                        nc.vector.tensor_copy(o, self.oaug[h01][0 : DK + 1, :])
                        osb.append(o)
                    for sub in range(MC // P):
                        otn2 = []
                        for h01 in range(2):
                            ot = mpps.tile([P, MC], f32, tag="mp", name="ot_t")
                            nc.tensor.transpose(
                                ot[:, 0 : DK + 1],
                                osb[h01][:, P * sub : P * (sub + 1)],
                                ident_f32[0 : DK + 1, 0 : DK + 1],
                            )
                            rc = rcpool.tile([P, 1], f32, tag="rc_c", name="rc_c_t")
                            nc.vector.reciprocal(rc, ot[:, DK : DK + 1])
                            otn = rcpool.tile([P, DK], f32, tag="otn", name="otn_t")
                            nc.vector.tensor_scalar_mul(otn, ot[:, 0:DK], rc)
                            otn2.append(otn)
                        if sub > 0:
                            emit_outproj_t(4 * c + sub - 1, scps, tail=True)
                        for h01 in range(2):
                            pt = oaps.tile([P, MC], f32, tag="oaug", name="pt_t")
                            nc.tensor.transpose(pt[0:DK, 0:P], otn2[h01], ident_f32)
                            dst = oT_sb[p][
                                64 * h01 : 64 * h01 + DK,
                                MC * c + P * sub : MC * c + P * (sub + 1),
                            ]
                            if h01:
                                nc.scalar.copy(dst, pt[0:DK, 0:P])
                            else:
                                nc.vector.tensor_copy(dst, pt[0:DK, 0:P])
                    emit_outproj_t(4 * c + 3, scps, tail=True)

            def emit_outproj_t(t, pool, tail=False):
                # out[m, e] partial for one m-tile t; oT-tile stationary
                # serving both 512-wide e-chunks, wo moving.
                ost = ostpool.tile([P, E], bf16, tag="ost", name="ost_t")
                for ec in range(E // MC):
                    op = pool.tile([P, 2 * MC], f32, tag="sc", name="op_t") if pool is scps \
                        else pool.tile([P, MC], f32, tag="mp", name="op_t")
                    for p in range(PAIRS):
                        nc.tensor.matmul(
                            op[:, 0:MC],
                            oT_sb[p][:, P * t : P * (t + 1)],
                            wo_sb[:, p, MC * ec : MC * (ec + 1)],
                            start=(p == 0),
                            stop=(p == PAIRS - 1),
                        )
                    if tail and ec:
                        # ACT is idle in the tail; share the cast work
                        nc.scalar.copy(ost[:, MC * ec : MC * (ec + 1)], op[:, 0:MC])
                    else:
                        nc.vector.tensor_copy(ost[:, MC * ec : MC * (ec + 1)], op[:, 0:MC])
                    if tail:
                        # drain the tail writes in halves across both idle
                        # queues (exps are done by now) to shorten the tail
                        eng = nc.scalar if (2 * t + ec) % 2 else nc.sync
                        eng.dma_start(
                            out=out[P * t : P * (t + 1), MC * ec : MC * (ec + 1)],
                            in_=ost[:, MC * ec : MC * (ec + 1)],
                        )
                if not tail:
                    nc.sync.dma_start(out=out[P * t : P * (t + 1), :], in_=ost)

            def emit_outproj(c):
                for t in range(4 * c, 4 * c + 4):
                    emit_outproj_t(t, mpps)

            groups = {(c, p): Group(c, p) for c in range(NMC) for p in range(PAIRS)}

            # One continuous PE stream: the scores/exp pipeline never drains
            # across group boundaries (scores stay 2 (g,j)-steps ahead of
            # attnV globally) and v-projection blocks / per-chunk output
            # projections slot in as independent PE filler.
            if causal:
                group_order = [(c, p) for c in range(NMC) for p in range(PAIRS)]
                # chunk-0 + chunk-1/pair-0 scores pre-issued: PE work while
                # xv streams in (their attnVs wait on v anyway)
                pre = PAIRS * 4 + 10  # scores steps emitted before the merge loop
                items = [("vp", 0, 4)]
                for c, p in group_order:
                    g = groups[(c, p)]
                    last = (c, p) == group_order[-1]
                    if last:
                        # the final group runs in ACT-lockstep (~23% PE
                        # slack per step): absorb the previous chunk's
                        # outproj t-tiles into that slack instead of
                        # running them as a serial block before it
                        for j in range(g.J):
                            items.append(("av", g, j))
                            if j % 4 == 2:
                                items.append(("opt", 4 * (NMC - 2) + j // 4))
                    else:
                        items += [("av", g, j) for j in range(g.J)]
                    items.append(("nmt" if last else "nm", g))
                    if (c, p) == (0, 1):
                        items.append(("vp", 4, 8))
                    elif (c, p) == (1, 0):
                        items += [("vp", 8, 12), ("op", 0)]
                    elif (c, p) == (1, 1):
                        items.append(("vp", 12, 16))
                    elif (c, p) == (2, 0):
                        items.append(("op", 1))
            else:
                group_order = [(c, p) for c in range(NMC) for p in range(PAIRS)]
                pre = 0
                items = [("vp", 0, NT)]
                for c, p in group_order:
                    g = groups[(c, p)]
                    items += [("av", g, j) for j in range(g.J)]
                    last = (c, p) == group_order[-1]
                    items.append(("nmt" if last else "nm", g))
                    if p == PAIRS - 1 and c > 0:
                        items.append(("op", c - 1))

            sc_steps = [
                (groups[(c, p)], j)
                for c, p in group_order
                for j in range(groups[(c, p)].J)
            ]
            # dummy transposes between the exp-paced pre-scores keep the
            # PE p-state up through the xv-arrival wall
            wt2 = mpps.tile([P, MC], f32, tag="mp", name="warm2_t")
            for idx, (gg, j) in enumerate(sc_steps[:pre]):
                gg.emit_scores(j)
                if idx >= 1:
                    for _ in range(4):
                        nc.tensor.transpose(wt2[:, 0:P], ident_f32, ident_f32)
            si = pre
            av_k = 0

            def pump_one():
                nonlocal si
                sg, sj = sc_steps[si]
                sg.emit_scores(sj)
                si += 1

            for item in items:
                if item[0] == "av":
                    while si < min(av_k + 3, len(sc_steps)):
                        pump_one()
                    item[1].emit_attnv(item[2])
                    av_k += 1
                elif item[0] == "nm":
                    item[1].emit_norm()
                elif item[0] == "nmt":
                    item[1].emit_norm_transpose()
                elif item[0] == "opt":
                    emit_outproj_t(item[1], mpps)
                elif item[0] == "vp":
                    # interleave future scores (ACT-paced) with v-proj
                    # n-tiles so the PE has work while xv streams in;
                    # si <= av_k + prpool_bufs - 1 guards probs-slot reuse
                    # against in-order deadlock (slot si-16's consumer must
                    # already be emitted).
                    for j in range(item[1], item[2]):
                        for _ in range(2):
                            if si < len(sc_steps) and si <= av_k + 17:
                                pump_one()
                        emit_vproj(j, j + 1)
                else:
                    emit_outproj(item[1])

    nc.compile()
    return nc


def _host_inputs(key, value, query, Wk, Wq, Wv, Wo, bq, bk, bv, bias_qk, bias_v):
    """Per-core input maps (host-side shard/transpose/cast — not timed)."""
    tri = np.triu(np.ones((P, P), np.float32)).astype(BF16)  # allowed: n<=m
    in_maps = []
    xT = {}
    for b in range(B):
        xT[("q", b)] = np.ascontiguousarray(query[b].T).astype(BF16)
        xT[("k", b)] = np.ascontiguousarray(key[b].T).astype(BF16)
        xT[("v", b)] = np.ascontiguousarray(value[b].T).astype(BF16)
    for c in range(NCORES):
        b, g = divmod(c, G)
        sl = slice(DKH * g, DKH * (g + 1))
        m = {
            "xqT": xT[("q", b)],
            "xkT": xT[("k", b)],
            "xvT": xT[("v", b)],
            "wkT": np.ascontiguousarray(Wk[sl].T).astype(BF16),
            "wqT": np.ascontiguousarray(Wq[sl].T).astype(BF16),
            "wvT": np.ascontiguousarray(Wv[sl].T).astype(BF16),
            "woT": np.ascontiguousarray(Wo[:, sl].T).astype(BF16),
            "dmask": tri,
        }
        if bias_qk:
            m["bq"] = np.ascontiguousarray(bq[sl].astype(np.float32).reshape(DKH, 1))
            m["bk"] = np.ascontiguousarray(bk[sl].astype(np.float32).reshape(DKH, 1))
        if bias_v:
            m["bv"] = np.ascontiguousarray(bv[sl].astype(np.float32).reshape(1, DKH))
        in_maps.append(m)
    return in_maps


def _numpy_fallback(key, value, query, mask, Wk, bk, Wq, bq, Wv, bv, Wo, bo):
    """Exact reference semantics in numpy (general-mask fallback)."""
    def proj(x, W, b):
        return x @ W.T + b

    k = proj(key, Wk, bk).reshape(B, S, H, DK).transpose(0, 2, 1, 3)
    q = proj(query, Wq, bq).reshape(B, S, H, DK).transpose(0, 2, 1, 3)
    v = proj(value, Wv, bv).reshape(B, S, H, DK).transpose(0, 2, 1, 3)
    scores = np.einsum("bhmd,bhnd->bhmn", q, k).astype(np.float32)
    scores = np.where(mask, scores, np.float32(-1e10)) * np.float32(SCALE)
    scores -= scores.max(axis=3, keepdims=True)
    e = np.exp(scores)
    attn = e / e.sum(axis=3, keepdims=True)
    o = np.einsum("bhmn,bhnv->bhmv", attn, v)
    o = o.transpose(0, 2, 1, 3).reshape(B, S, E)
    return (o @ Wo.T + bo).astype(np.float32)


_program_cache = {}


def kernel(key, value, query, mask, Wk, bk, Wq, bq, Wv, bv, Wo, bo):
    key = np.asarray(key, np.float32)
    value = np.asarray(value, np.float32)
    query = np.asarray(query, np.float32)
    mask = np.asarray(mask)
    Wk, bk = np.asarray(Wk, np.float32), np.asarray(bk, np.float32)
    Wq, bq = np.asarray(Wq, np.float32), np.asarray(bq, np.float32)
    Wv, bv = np.asarray(Wv, np.float32), np.asarray(bv, np.float32)
    Wo, bo = np.asarray(Wo, np.float32), np.asarray(bo, np.float32)

    m2 = mask.reshape(B, S, S) if mask.size == B * S * S else None
    causal = m2 is not None and all(
        np.array_equal(m2[b], np.tril(np.ones((S, S), bool))) for b in range(B)
    )
    allones = m2 is not None and bool(mask.all())
    if not causal and not allones:
        return _numpy_fallback(key, value, query, mask, Wk, bk, Wq, bq, Wv, bv, Wo, bo)

    if causal:
        chunk_ntiles = tuple(4 * (c + 1) for c in range(NMC))
    else:
        chunk_ntiles = tuple(NT for _ in range(NMC))

    bias_qk = bool(np.any(bq) or np.any(bk))
    bias_v = bool(np.any(bv))

    pkey = (chunk_ntiles, causal, bias_qk, bias_v)
    if pkey not in _program_cache:
        _program_cache[pkey] = _build_program(chunk_ntiles, causal, bias_qk, bias_v)
    nc = _program_cache[pkey]

    from concourse.bass_utils import run_bass_kernel_spmd

    in_maps = _host_inputs(key, value, query, Wk, Wq, Wv, Wo, bq, bk, bv, bias_qk, bias_v)
    res = run_bass_kernel_spmd(nc, in_maps, core_ids=list(range(NCORES)))

    outp = np.zeros((B, S, E), np.float32)
    for c in range(NCORES):
        outp[c // G] += np.asarray(res.results[c]["out"], np.float32)
    outp += bo.astype(np.float32)
    return outp


# revision 41
# speedup vs baseline: 1.0356x; 1.0356x over previous
"""Trainium2 Bass kernel: 16-head MHA (B=2, S=2048, E=1024) on 8 NeuronCores.

Sharding: core c = (batch b = c // 4, head-group g = c % 4); each core runs
4 heads of one batch (data parallel on B x tensor parallel on heads).  The
output projection is row-sharded: each core produces a partial [S, E] bf16
output; the host sums the 4 head-group partials per batch (f32) and adds bo.

Device pipeline per core (all matmul operands bf16, fp32 PSUM accumulation).
The PE instruction stream is one continuous sequence ordered to never starve
(matmuls back-to-back keep the PE at its top p-state):

  k-proj -> q-proj -> scores(chunk0, both pairs)      [pre-issued: fills the
                                                       xv DMA-arrival gap]
  -> v[n-tiles 0:4] -> attnV(chunk0) -> v[4:8]
  -> attn(c1,p0) -> v[8:12] -> outproj(c0) -> attn(c1,p1) -> v[12:16]
  -> attn(c2,p0) -> outproj(c1) -> attn(c2,p1)
  -> attn(c3,p0) -> outproj(c2) -> attn(c3,p1) -> outproj(c3)

All input DMAs issue on the sync queue in priority order (wk/xk per e-tile
interleaved, then wqv, xq, xv) so each projection's operands stream in just
ahead of the PE.  v is produced directly in [n, dv] layout (stationary x
tile, moving wv) -- no PE transposes.  The softmax denominator rides as a
ones-column in v_aug; normalization bounces the denominator row through DRAM
(reshape to [128, 8] for a wide reciprocal, broadcast back with a 0-stride
partition AP), batched over both head-halves per group.
"""

import numpy as np
import ml_dtypes

B, S, E = 2, 2048, 1024
H, DK = 16, 64
NCORES = 8
G = 4                 # head-groups (tensor parallel degree)
NH = H // G           # heads per core = 4
DKH = NH * DK         # 256 head dims per core
P = 128
MC = 512              # m-chunk (psum bank width in f32)
NMC = S // MC         # 4 m-chunks
NT = S // P           # 16 n-tiles (and m-tiles)
ET = E // P           # 8 e-tiles
PAIRS = NH // 2       # 2 head pairs per core
BF16 = ml_dtypes.bfloat16
SCALE = float(1.0 / np.sqrt(np.float32(DK)))


def _build_program(chunk_ntiles, causal, bias_qk, bias_v):
    """Build the (SPMD, shared across all 8 cores) Bass program.

    chunk_ntiles[c] = number of 128-wide n-tiles to process for m-chunk c.
    causal: apply diagonal-tile masking (memset + tri multiply).
    """
    from contextlib import ExitStack

    import concourse.bass as bass
    import concourse.tile as tile
    from concourse import bacc, mybir
    from concourse.masks import make_identity

    f32 = mybir.dt.float32
    bf16 = mybir.dt.bfloat16
    Exp = mybir.ActivationFunctionType.Exp

    nc = bacc.Bacc(
        "TRN2",
        target_bir_lowering=False,
        debug=False,
        enable_asserts=False,
        num_devices=NCORES,
    )

    # ---- DRAM I/O ----
    xqT = nc.dram_tensor("xqT", [E, S], bf16, kind="ExternalInput").ap()
    xkT = nc.dram_tensor("xkT", [E, S], bf16, kind="ExternalInput").ap()
    xvT = nc.dram_tensor("xvT", [E, S], bf16, kind="ExternalInput").ap()
    wkT = nc.dram_tensor("wkT", [E, DKH], bf16, kind="ExternalInput").ap()
    wqT = nc.dram_tensor("wqT", [E, DKH], bf16, kind="ExternalInput").ap()
    wvT = nc.dram_tensor("wvT", [E, DKH], bf16, kind="ExternalInput").ap()
    woT = nc.dram_tensor("woT", [DKH, E], bf16, kind="ExternalInput").ap()
    dmask = nc.dram_tensor("dmask", [P, P], bf16, kind="ExternalInput").ap()
    if bias_qk:
        bqd = nc.dram_tensor("bq", [DKH, 1], f32, kind="ExternalInput").ap()
        bkd = nc.dram_tensor("bk", [DKH, 1], f32, kind="ExternalInput").ap()
    if bias_v:
        bvd = nc.dram_tensor("bv", [1, DKH], f32, kind="ExternalInput").ap()
    out = nc.dram_tensor("out", [S, E], bf16, kind="ExternalOutput").ap()

    with tile.TileContext(nc) as tc, ExitStack() as ctx:
        const = ctx.enter_context(tc.tile_pool(name="const", bufs=1))
        xpool = ctx.enter_context(tc.tile_pool(name="xpool", bufs=1))
        wpool = ctx.enter_context(tc.tile_pool(name="wpool", bufs=1))
        qkpool = ctx.enter_context(tc.tile_pool(name="qkpool", bufs=1))
        vpool = ctx.enter_context(tc.tile_pool(name="vpool", bufs=1))
        prpool = ctx.enter_context(tc.tile_pool(name="prpool", bufs=16))
        rcpool = ctx.enter_context(tc.tile_pool(name="rcpool", bufs=4))
        bcpool = ctx.enter_context(tc.tile_pool(name="bcpool", bufs=2))
        otpool = ctx.enter_context(tc.tile_pool(name="otpool", bufs=1))
        ostpool = ctx.enter_context(tc.tile_pool(name="ostpool", bufs=4))

        # ---- all input DMAs on the sync queue, in priority order ----
        # (one issuing queue => transfers start in issue order, so each
        # projection's operands arrive just ahead of the PE's need: the DMA
        # engines drain the queue FIFO at full HBM bandwidth.)
        wk_sb = wpool.tile([P, ET, DKH], bf16, tag="wk")
        wq_sb = wpool.tile([P, ET, DKH], bf16, tag="wq")
        wv_sb = wpool.tile([P, ET, DKH], bf16, tag="wv")
        wo_sb = wpool.tile([P, PAIRS, E], bf16, tag="wo")
        xk_sb = xpool.tile([P, ET, S], bf16, tag="xk")
        xq_sb = xpool.tile([P, ET, S], bf16, tag="xq")
        xv_sb = xpool.tile([P, ET, S], bf16, tag="xv")
        xkr = xkT.rearrange("(t p) s -> p t s", p=P)
        xqr = xqT.rearrange("(t p) s -> p t s", p=P)
        xvr = xvT.rearrange("(t p) s -> p t s", p=P)
        wkr = wkT.rearrange("(t p) d -> p t d", p=P)
        wqr = wqT.rearrange("(t p) d -> p t d", p=P)
        wvr = wvT.rearrange("(t p) d -> p t d", p=P)
        for i in range(ET):
            nc.sync.dma_start(out=wk_sb[:, i, :], in_=wkr[:, i, :])
            if i == 0:
                # split so the first projection matmul's [0:512] region
                # dependency fires after a quarter of the bytes
                nc.sync.dma_start(out=xk_sb[:, 0, 0:MC], in_=xkr[:, 0, 0:MC])
                nc.sync.dma_start(out=xk_sb[:, 0, MC:S], in_=xkr[:, 0, MC:S])
            else:
                nc.sync.dma_start(out=xk_sb[:, i, :], in_=xkr[:, i, :])
        for i in range(ET):
            nc.sync.dma_start(out=wq_sb[:, i, :], in_=wqr[:, i, :])
            if i == ET - 1:
                # the last e-tile gates the whole q projection: split it so
                # the first m-chunk's accumulation can finish sooner
                nc.sync.dma_start(out=xq_sb[:, i, 0:MC], in_=xqr[:, i, 0:MC])
                nc.sync.dma_start(out=xq_sb[:, i, MC:S], in_=xqr[:, i, MC:S])
            else:
                nc.sync.dma_start(out=xq_sb[:, i, :], in_=xqr[:, i, :])
        nc.sync.dma_start(out=wv_sb, in_=wvr)
        for i in range(ET):
            nc.sync.dma_start(out=xv_sb[:, i, :], in_=xvr[:, i, :])
        for p in range(PAIRS):
            nc.sync.dma_start(out=wo_sb[:, p, :], in_=woT[P * p : P * (p + 1), :])
        dmask_sb = const.tile([P, P], bf16, tag="dmask")
        nc.sync.dma_start(out=dmask_sb, in_=dmask)
        ident_f32 = const.tile([P, P], f32, tag="ident")
        make_identity(nc, ident_f32)
        if bias_qk:
            bq_sb = const.tile([P, PAIRS], f32, tag="bq")
            nc.sync.dma_start(out=bq_sb, in_=bqd.rearrange("(t p) o -> p (t o)", p=P))
            bk_sb = const.tile([P, PAIRS], f32, tag="bk")
            nc.sync.dma_start(out=bk_sb, in_=bkd.rearrange("(t p) o -> p (t o)", p=P))
        if bias_v:
            bv_sb = const.tile([P, 2 * DKH], f32, tag="bv")
            nc.sync.dma_start(
                out=bv_sb,
                in_=bass.AP(
                    tensor=bvd.tensor,
                    offset=bvd.offset,
                    ap=[[0, P], [1, DKH]],
                ),
            )

        # persistent activation tiles
        qT_sb = [qkpool.tile([P, S], bf16, tag=f"qT{p}", name=f"qT_sb{p}") for p in range(PAIRS)]
        kT_sb = [qkpool.tile([P, S], bf16, tag=f"kT{p}", name=f"kT_sb{p}") for p in range(PAIRS)]
        vaug_sb = [vpool.tile([P, NH, DK + 1], bf16, tag=f"va{j}", name=f"vaug_sb{j}") for j in range(NT)]
        oT_sb = [otpool.tile([P, S], bf16, tag=f"oT{p}", name=f"oT_sb{p}") for p in range(PAIRS)]

        # PE p-state warm-up: the tensor engine's clock ramps only while
        # continuously busy, so burn the DMA-arrival wait on dummy
        # transposes to hit full clock by the first real matmul
        with tc.tile_pool(name="warm_ps", bufs=1, space="PSUM") as wps:
            wt = wps.tile([P, P], f32, tag="warm", name="warm_t")
            for _ in range(33):
                nc.tensor.transpose(wt, ident_f32, ident_f32)

        # ---- stage 1: q/k projections (weight-tile stationary) ----
        # both pairs interleaved per e-tile (8 psum banks) so a single
        # e-tile's DMA arrival unblocks 8 back-to-back matmuls
        with tc.tile_pool(name="pj_ps", bufs=8, space="PSUM") as pjps:
            for dst, w_sb, x_sb, bias in (
                (kT_sb, wk_sb, xk_sb, bk_sb if bias_qk else None),
                (qT_sb, wq_sb, xq_sb, bq_sb if bias_qk else None),
            ):
                ps = [
                    [pjps.tile([P, MC], f32, tag="qk", name="ps_qk") for _ in range(NMC)]
                    for _ in range(PAIRS)
                ]
                for i in range(ET):
                    for p in range(PAIRS):
                        for c in range(NMC):
                            nc.tensor.matmul(
                                ps[p][c],
                                w_sb[:, i, P * p : P * (p + 1)],
                                x_sb[:, i, MC * c : MC * (c + 1)],
                                start=(i == 0),
                                stop=(i == ET - 1),
                            )
                # evict on DVE (idle in this phase) so the ACT engine can
                # start the pre-issued score exps as soon as they appear
                for p in range(PAIRS):
                    for c in range(NMC):
                        dslice = dst[p][:, MC * c : MC * (c + 1)]
                        if bias is not None:
                            nc.vector.tensor_scalar_add(dslice, ps[p][c], bias[:, p : p + 1])
                        else:
                            nc.vector.tensor_copy(dslice, ps[p][c])

        # ---- stages 2+3 pools: 4 (scores) + 2 (o_aug) + 2 (v-proj/outproj)
        # psum banks = 8 total.  v-projection n-tile blocks, attention groups
        # and per-chunk output projections are interleaved in ONE PE stream.
        with (
            tc.tile_pool(name="sc_ps", bufs=2, space="PSUM") as scps,
            tc.tile_pool(name="oa_ps", bufs=2, space="PSUM") as oaps,
            tc.tile_pool(name="mp_ps", bufs=2, space="PSUM") as mpps,
            tc.tile_pool(name="rc_dram", bufs=4, space="DRAM") as rcdram,
        ):

            def emit_vproj(j0, j1):
                # v directly in [n, dv] layout: stationary x-tile [e, n],
                # moving wv [e, dv] accumulated over e-tiles.
                for j in range(j0, j1):
                    ps = mpps.tile([P, MC], f32, tag="mp", name="ps_v")
                    for i in range(ET):
                        nc.tensor.matmul(
                            ps[:, 0:DKH],
                            xv_sb[:, i, P * j : P * (j + 1)],
                            wv_sb[:, i, :],
                            start=(i == 0),
                            stop=(i == ET - 1),
                        )
                    ps3 = ps[:, 0:DKH].rearrange("n (h d) -> n h d", h=NH)
                    if bias_v:
                        bv3 = bv_sb.rearrange("n (h d) -> n h d", h=NH)
                        nc.vector.tensor_add(vaug_sb[j][:, :, 0:DK], ps3, bv3)
                    else:
                        nc.vector.tensor_copy(vaug_sb[j][:, :, 0:DK], ps3)
                    nc.vector.memset(vaug_sb[j][:, :, DK : DK + 1], 1.0)

            class Group:
                """Attention for one (m-chunk c, head-pair p)."""

                def __init__(self, c, p):
                    self.c, self.p = c, p
                    self.J = chunk_ntiles[c]
                    self.probs = [None] * self.J
                    self.oaug = None

                def emit_scores(self, j):
                    c, p = self.c, self.p
                    off = P * (j - 4 * c) if (causal and j >= 4 * c) else 0
                    sc = scps.tile([P, 2 * MC], f32, tag="sc", name="sc_ps_t")
                    for h01 in range(2):
                        nc.tensor.matmul(
                            sc[:, MC * h01 + off : MC * (h01 + 1)],
                            kT_sb[p][64 * h01 : 64 * (h01 + 1), P * j : P * (j + 1)],
                            qT_sb[p][64 * h01 : 64 * (h01 + 1), MC * c + off : MC * (c + 1)],
                            start=True,
                            stop=True,
                        )
                    probs = prpool.tile([P, 2 * MC], bf16, tag="probs", name="probs_t")
                    sc3 = sc.rearrange("p (u m) -> p u m", u=2)
                    pr3 = probs.rearrange("p (u m) -> p u m", u=2)
                    nc.scalar.activation(
                        pr3[:, :, off:MC], sc3[:, :, off:MC], Exp, bias=0.0, scale=SCALE
                    )
                    if causal and j >= 4 * c:
                        for h01 in range(2):
                            base = MC * h01 + off
                            nc.gpsimd.tensor_mul(
                                probs[:, base : base + P],
                                probs[:, base : base + P],
                                dmask_sb,
                            )
                    self.probs[j] = (probs, off)

                def emit_attnv(self, j):
                    if self.oaug is None:
                        self.oaug = [
                            oaps.tile([P, MC], f32, tag="oaug", name=f"oaug{h01}")
                            for h01 in range(2)
                        ]
                    probs, off = self.probs[j]
                    for h01 in range(2):
                        h = 2 * self.p + h01
                        nc.tensor.matmul(
                            self.oaug[h01][0 : DK + 1, off:MC],
                            vaug_sb[j][:, h, :],
                            probs[:, MC * h01 + off : MC * (h01 + 1)],
                            start=(j == 0),
                            stop=(j == self.J - 1),
                        )

                def emit_norm(self):
                    # evict o_aug to SBUF (frees the PSUM slot), then divide
                    # rows 0:64 by row 64 (the ridden-along softmax denom).
                    # The [1, 2*MC] denom row would be a single-lane DVE op;
                    # bounce through DRAM: reshape to [128, 8] for a wide
                    # reciprocal, then re-read with a 0-stride partition AP
                    # as a [64, 2*MC] broadcast.  Both head-halves batched.
                    c, p = self.c, self.p
                    osb = []
                    for h01 in range(2):
                        o = rcpool.tile([DK + 1, MC], f32, tag="osb", name="osb_t")
                        nc.vector.tensor_copy(o, self.oaug[h01][0 : DK + 1, :])
                        osb.append(o)
                    den_d = rcdram.tile([2, MC], f32, tag="den_d", name="den_d_t")
                    for h01 in range(2):
                        nc.sync.dma_start(
                            out=den_d[h01 : h01 + 1, :], in_=osb[h01][DK : DK + 1, :]
                        )
                    den_q = rcpool.tile([P, 2 * MC // P], f32, tag="den_q", name="den_q_t")
                    nc.sync.dma_start(
                        out=den_q,
                        in_=bass.AP(
                            tensor=den_d.tensor,
                            offset=den_d.offset,
                            ap=[[2 * MC // P, P], [1, 2 * MC // P]],
                        ),
                    )
                    rcq = rcpool.tile([P, 2 * MC // P], f32, tag="rcq", name="rcq_t")
                    nc.vector.reciprocal(rcq, den_q)
                    rcd = rcdram.tile([1, 2 * MC], f32, tag="rcd", name="rcd_t")
                    nc.sync.dma_start(
                        out=bass.AP(
                            tensor=rcd.tensor,
                            offset=rcd.offset,
                            ap=[[2 * MC // P, P], [1, 2 * MC // P]],
                        ),
                        in_=rcq,
                    )
                    bc = bcpool.tile([64, 2 * MC], f32, tag="bc", name="bc_t")
                    nc.sync.dma_start(
                        out=bc,
                        in_=bass.AP(
                            tensor=rcd.tensor,
                            offset=rcd.offset,
                            ap=[[0, 64]] + [list(a) for a in rcd.ap[1:]],
                        ),
                    )
                    for h01 in range(2):
                        nc.vector.tensor_mul(
                            oT_sb[p][64 * h01 : 64 * (h01 + 1), MC * c : MC * (c + 1)],
                            osb[h01][0:DK, :],
                            bc[:, MC * h01 : MC * (h01 + 1)],
                        )

                def emit_norm_transpose(self):
                    # all-on-chip normalization (no DMA hops): PE-transpose
                    # each [65, 128] block of o_aug so the denominator becomes
                    # a [128, 1] column, wide-reciprocal it, per-partition
                    # multiply, transpose back.  Used for the final group,
                    # whose normalization latency is tail-exposed; the final
                    # chunk's outproj t-tiles slot between the forward and
                    # back transposes as PE filler for the DVE latency.
                    c, p = self.c, self.p
                    osb = []
                    for h01 in range(2):
                        o = rcpool.tile([DK + 1, MC], f32, tag="osb", name="osb_t")
                        nc.vector.tensor_copy(o, self.oaug[h01][0 : DK + 1, :])
                        osb.append(o)
                    for sub in range(MC // P):
                        otn2 = []
                        for h01 in range(2):
                            ot = mpps.tile([P, MC], f32, tag="mp", name="ot_t")
                            nc.tensor.transpose(
                                ot[:, 0 : DK + 1],
                                osb[h01][:, P * sub : P * (sub + 1)],
                                ident_f32[0 : DK + 1, 0 : DK + 1],
                            )
                            rc = rcpool.tile([P, 1], f32, tag="rc_c", name="rc_c_t")
                            nc.vector.reciprocal(rc, ot[:, DK : DK + 1])
                            otn = rcpool.tile([P, DK], f32, tag="otn", name="otn_t")
                            nc.vector.tensor_scalar_mul(otn, ot[:, 0:DK], rc)
                            otn2.append(otn)
                        if sub > 0:
                            emit_outproj_t(4 * c + sub - 1, scps, tail=True)
                        for h01 in range(2):
                            pt = oaps.tile([P, MC], f32, tag="oaug", name="pt_t")
                            nc.tensor.transpose(pt[0:DK, 0:P], otn2[h01], ident_f32)
                            dst = oT_sb[p][
                                64 * h01 : 64 * h01 + DK,
                                MC * c + P * sub : MC * c + P * (sub + 1),
                            ]
                            if h01:
                                nc.scalar.copy(dst, pt[0:DK, 0:P])
                            else:
                                nc.vector.tensor_copy(dst, pt[0:DK, 0:P])
                    emit_outproj_t(4 * c + 3, scps, tail=True)

            def emit_outproj_t(t, pool, tail=False):
                # out[m, e] partial for one m-tile t; oT-tile stationary
                # serving both 512-wide e-chunks, wo moving.
                ost = ostpool.tile([P, E], bf16, tag="ost", name="ost_t")
                for ec in range(E // MC):
                    op = pool.tile([P, 2 * MC], f32, tag="sc", name="op_t") if pool is scps \
                        else pool.tile([P, MC], f32, tag="mp", name="op_t")
                    for p in range(PAIRS):
                        nc.tensor.matmul(
                            op[:, 0:MC],
                            oT_sb[p][:, P * t : P * (t + 1)],
                            wo_sb[:, p, MC * ec : MC * (ec + 1)],
                            start=(p == 0),
                            stop=(p == PAIRS - 1),
                        )
                    if tail and ec:
                        # ACT is idle in the tail; share the cast work
                        nc.scalar.copy(ost[:, MC * ec : MC * (ec + 1)], op[:, 0:MC])
                    else:
                        nc.vector.tensor_copy(ost[:, MC * ec : MC * (ec + 1)], op[:, 0:MC])
                    if tail:
                        # drain the tail writes in halves across both idle
                        # queues (exps are done by now) to shorten the tail
                        eng = nc.scalar if (2 * t + ec) % 2 else nc.sync
                        eng.dma_start(
                            out=out[P * t : P * (t + 1), MC * ec : MC * (ec + 1)],
                            in_=ost[:, MC * ec : MC * (ec + 1)],
                        )
                if not tail:
                    nc.sync.dma_start(out=out[P * t : P * (t + 1), :], in_=ost)

            def emit_outproj(c):
                for t in range(4 * c, 4 * c + 4):
                    emit_outproj_t(t, mpps)

            groups = {(c, p): Group(c, p) for c in range(NMC) for p in range(PAIRS)}

            # One continuous PE stream: the scores/exp pipeline never drains
            # across group boundaries (scores stay 2 (g,j)-steps ahead of
            # attnV globally) and v-projection blocks / per-chunk output
            # projections slot in as independent PE filler.
            if causal:
                group_order = [(c, p) for c in range(NMC) for p in range(PAIRS)]
                # chunk-0 + chunk-1/pair-0 scores pre-issued: PE work while
                # xv streams in (their attnVs wait on v anyway)
                pre = PAIRS * 4 + 8  # scores steps emitted before the merge loop
                items = [("vp", 0, 4)]
                for c, p in group_order:
                    g = groups[(c, p)]
                    last = (c, p) == group_order[-1]
                    if last:
                        # the final group runs in ACT-lockstep (~23% PE
                        # slack per step): absorb the previous chunk's
                        # outproj t-tiles into that slack instead of
                        # running them as a serial block before it
                        for j in range(g.J):
                            items.append(("av", g, j))
                            if j % 4 == 2:
                                items.append(("opt", 4 * (NMC - 2) + j // 4))
                    else:
                        items += [("av", g, j) for j in range(g.J)]
                    items.append(("nmt" if last else "nm", g))
                    if (c, p) == (0, 1):
                        items.append(("vp", 4, 8))
                    elif (c, p) == (1, 0):
                        items += [("vp", 8, 12), ("op", 0)]
                    elif (c, p) == (1, 1):
                        items.append(("vp", 12, 16))
                    elif (c, p) == (2, 0):
                        items.append(("op", 1))
            else:
                group_order = [(c, p) for c in range(NMC) for p in range(PAIRS)]
                pre = 0
                items = [("vp", 0, NT)]
                for c, p in group_order:
                    g = groups[(c, p)]
                    items += [("av", g, j) for j in range(g.J)]
                    last = (c, p) == group_order[-1]
                    items.append(("nmt" if last else "nm", g))
                    if p == PAIRS - 1 and c > 0:
                        items.append(("op", c - 1))

            sc_steps = [
                (groups[(c, p)], j)
                for c, p in group_order
                for j in range(groups[(c, p)].J)
            ]
            for gg, j in sc_steps[:pre]:
                gg.emit_scores(j)
            si = pre
            av_k = 0

            def pump_one():
                nonlocal si
                sg, sj = sc_steps[si]
                sg.emit_scores(sj)
                si += 1

            for item in items:
                if item[0] == "av":
                    while si < min(av_k + 4, len(sc_steps)):
                        pump_one()
                    item[1].emit_attnv(item[2])
                    av_k += 1
                elif item[0] == "nm":
                    item[1].emit_norm()
                elif item[0] == "nmt":
                    item[1].emit_norm_transpose()
                elif item[0] == "opt":
                    emit_outproj_t(item[1], mpps)
                elif item[0] == "vp":
                    # interleave future scores (ACT-paced) with v-proj
                    # n-tiles so the PE has work while xv streams in;
                    # si <= av_k + prpool_bufs - 1 guards probs-slot reuse
                    # against in-order deadlock (slot si-16's consumer must
                    # already be emitted).
                    for j in range(item[1], item[2]):
                        for _ in range(2):
                            if si < len(sc_steps) and si <= av_k + 15:
                                pump_one()
                        emit_vproj(j, j + 1)
                else:
                    emit_outproj(item[1])

    nc.compile()
    return nc


def _host_inputs(key, value, query, Wk, Wq, Wv, Wo, bq, bk, bv, bias_qk, bias_v):
    """Per-core input maps (host-side shard/transpose/cast — not timed)."""
    tri = np.triu(np.ones((P, P), np.float32)).astype(BF16)  # allowed: n<=m
    in_maps = []
    xT = {}
    for b in range(B):
        xT[("q", b)] = np.ascontiguousarray(query[b].T).astype(BF16)
        xT[("k", b)] = np.ascontiguousarray(key[b].T).astype(BF16)
        xT[("v", b)] = np.ascontiguousarray(value[b].T).astype(BF16)
    for c in range(NCORES):
        b, g = divmod(c, G)
        sl = slice(DKH * g, DKH * (g + 1))
        m = {
            "xqT": xT[("q", b)],
            "xkT": xT[("k", b)],
            "xvT": xT[("v", b)],
            "wkT": np.ascontiguousarray(Wk[sl].T).astype(BF16),
            "wqT": np.ascontiguousarray(Wq[sl].T).astype(BF16),
            "wvT": np.ascontiguousarray(Wv[sl].T).astype(BF16),
            "woT": np.ascontiguousarray(Wo[:, sl].T).astype(BF16),
            "dmask": tri,
        }
        if bias_qk:
            m["bq"] = np.ascontiguousarray(bq[sl].astype(np.float32).reshape(DKH, 1))
            m["bk"] = np.ascontiguousarray(bk[sl].astype(np.float32).reshape(DKH, 1))
        if bias_v:
            m["bv"] = np.ascontiguousarray(bv[sl].astype(np.float32).reshape(1, DKH))
        in_maps.append(m)
    return in_maps


def _numpy_fallback(key, value, query, mask, Wk, bk, Wq, bq, Wv, bv, Wo, bo):
    """Exact reference semantics in numpy (general-mask fallback)."""
    def proj(x, W, b):
        return x @ W.T + b

    k = proj(key, Wk, bk).reshape(B, S, H, DK).transpose(0, 2, 1, 3)
    q = proj(query, Wq, bq).reshape(B, S, H, DK).transpose(0, 2, 1, 3)
    v = proj(value, Wv, bv).reshape(B, S, H, DK).transpose(0, 2, 1, 3)
    scores = np.einsum("bhmd,bhnd->bhmn", q, k).astype(np.float32)
    scores = np.where(mask, scores, np.float32(-1e10)) * np.float32(SCALE)
    scores -= scores.max(axis=3, keepdims=True)
    e = np.exp(scores)
    attn = e / e.sum(axis=3, keepdims=True)
    o = np.einsum("bhmn,bhnv->bhmv", attn, v)
    o = o.transpose(0, 2, 1, 3).reshape(B, S, E)
    return (o @ Wo.T + bo).astype(np.float32)


_program_cache = {}


def kernel(key, value, query, mask, Wk, bk, Wq, bq, Wv, bv, Wo, bo):
    key = np.asarray(key, np.float32)
    value = np.asarray(value, np.float32)
    query = np.asarray(query, np.float32)
    mask = np.asarray(mask)
    Wk, bk = np.asarray(Wk, np.float32), np.asarray(bk, np.float32)
    Wq, bq = np.asarray(Wq, np.float32), np.asarray(bq, np.float32)
    Wv, bv = np.asarray(Wv, np.float32), np.asarray(bv, np.float32)
    Wo, bo = np.asarray(Wo, np.float32), np.asarray(bo, np.float32)

    m2 = mask.reshape(B, S, S) if mask.size == B * S * S else None
    causal = m2 is not None and all(
        np.array_equal(m2[b], np.tril(np.ones((S, S), bool))) for b in range(B)
    )
    allones = m2 is not None and bool(mask.all())
    if not causal and not allones:
        return _numpy_fallback(key, value, query, mask, Wk, bk, Wq, bq, Wv, bv, Wo, bo)

    if causal:
        chunk_ntiles = tuple(4 * (c + 1) for c in range(NMC))
    else:
        chunk_ntiles = tuple(NT for _ in range(NMC))

    bias_qk = bool(np.any(bq) or np.any(bk))
    bias_v = bool(np.any(bv))

    pkey = (chunk_ntiles, causal, bias_qk, bias_v)
    if pkey not in _program_cache:
        _program_cache[pkey] = _build_program(chunk_ntiles, causal, bias_qk, bias_v)
    nc = _program_cache[pkey]

    from concourse.bass_utils import run_bass_kernel_spmd

    in_maps = _host_inputs(key, value, query, Wk, Wq, Wv, Wo, bq, bk, bv, bias_qk, bias_v)
    res = run_bass_kernel_spmd(nc, in_maps, core_ids=list(range(NCORES)))

    outp = np.zeros((B, S, E), np.float32)
    for c in range(NCORES):
        outp[c // G] += np.asarray(res.results[c]["out"], np.float32)
    outp += bo.astype(np.float32)
    return outp
